# revision 18
# baseline (speedup 1.0000x reference)
"""Trainium2 Bass kernel for nn_AttentionLayer (Bahdanau-style attention scorer).

Math (per batch b):
    x   = concat([a, broadcast(s)], -1)            # [Tx, Da+Ds]
    h   = relu(x @ W1 + b1)                        # [Tx, H]
    e   = tanh(h @ W2 + b2)                        # [Tx, 1]
    al  = softmax(e, axis=Tx)
    ctx = al^T @ a                                 # [1, Da]

Since e = tanh(.) is in [-1, 1], softmax needs no max subtraction.

Sharding: data-parallel over B across 8 cores (8 batches each).

v3 design — 1.5-layout streaming (12.6 MB/core vs the baseline's 16.8):
`a` ships transposed+bf16 (aT: features on partitions) for ALL batches —
the score matmuls consume it directly — and ADDITIONALLY in natural
layout (a_nat) for batches 4-7 only, queued AFTER the aT stream.

Context path is split by batch to balance engines:
  - batches 0-3 ("vector route", softmax output available early): PE
    broadcasts the batch's softmax row p into PSUM fp32 (K=1 matmuls
    with a ones stationary); DVE multiplies aT slices against it into a
    bf16 scratch; the time reduction runs as DVE tensor_reduce (k=0) and
    ACT Identity+accum_out (k=1 of batches 2-3) to share the load.
    (The fused DVE tensor_tensor_reduce op crashes this runtime, and
    gpsimd cannot touch PSUM — hence this split.)
  - batches 4-7 ("PE route"): p transposed to time-major via PE
    transposes, then ctx = sum_n pT_n^T @ a_nat_n as 4-way column-tiled
    quads accumulating at PSUM partitions 0/32/64/96 (quarters summed on
    host), exactly while their a_nat tiles stream in.

Scores: mm1 as column-tiled PAIRS (two 512-wide time slices through
array cols 0-63/64-127); relu+s-term bias on ACT (s @ W1s + b1 is
time-independent, folded into the bias).  mm2 writes each batch's e row
independently at PSUM partition 32j via tile_position column quadrants
(M=32 stationary with W2 in col 0 zero-pads rows 32j+1..32j+31 so
tanh/exp can run over a contiguous initialized block).  Softmax groups
(2,2,4); tanh/exp in 1024-halves on ACT with accum_out partial
denominators (summed on host; division on host).

Host-side preprocessing (transpose/cast/shard + final division) is numpy.
"""

import os
import sys

import numpy as np

for _p in ("/opt/trn_rl_repo", "/root/.axon_site/_ro/trn_rl_repo"):
    if os.path.isdir(_p) and _p not in sys.path:
        sys.path.insert(0, _p)

import ml_dtypes  # noqa: E402

import concourse.bacc as bacc  # noqa: E402
import concourse.bass as bass  # noqa: E402
import concourse.mybir as mybir  # noqa: E402
import concourse.tile as tile  # noqa: E402

BF16 = mybir.dt.bfloat16
F32 = mybir.dt.float32
NPBF16 = ml_dtypes.bfloat16
AF = mybir.ActivationFunctionType
ALU = mybir.AluOpType
PSUM = bass.MemorySpace.PSUM

NCORES = 8
B, TX, DA, DS, H = 64, 2048, 256, 256, 50
BPC = B // NCORES  # batches per core
NT = TX // 128  # 128-wide time chunks (PE-route ctx)
NTS = TX // 512  # 512-wide time slices
KD = DA // 128  # feature chunks of a

GROUPS = [(0, 2), (2, 2), (4, 4)]  # (first batch, size) per softmax group
GB = max(sz for _, sz in GROUPS)
PR = 32 * GB  # e/p rows live at partitions 0/32/64/96 (base-partition rule)

NVEC = 4  # batches 0..NVEC-1 take the vector route; the rest the PE route
ACT_K1 = (2, 3)  # vector-route batches whose k=1 reduction runs on ACT


def build_nc():
    """Build the (SPMD-identical) single-core Bass program."""
    nc = bacc.Bacc(
        "TRN2", target_bir_lowering=False, debug=False, num_devices=NCORES
    )

    aT = nc.dram_tensor("aT", [BPC, 128, KD, TX], BF16, kind="ExternalInput")
    a_nat = nc.dram_tensor(
        "a_nat", [BPC - NVEC, 128, NT, DA], BF16, kind="ExternalInput"
    )
    w1a = nc.dram_tensor("w1a", [128, KD, 64], BF16, kind="ExternalInput")
    w1s = nc.dram_tensor("w1s", [128, KD, H], F32, kind="ExternalInput")
    sT = nc.dram_tensor("sT", [128, KD, BPC], F32, kind="ExternalInput")
    # b1c / w2c carry two copies of their payload: partition rows 0-49 and
    # 64-113 (the two tile_position column/row groups used below).
    b1c = nc.dram_tensor("b1c", [128, 1], F32, kind="ExternalInput")
    w2c = nc.dram_tensor("w2c", [128, 32], BF16, kind="ExternalInput")
    b2c = nc.dram_tensor("b2c", [PR, 1], F32, kind="ExternalInput")
    ones = nc.dram_tensor("ones", [PR, 128], BF16, kind="ExternalInput")
    idm = nc.dram_tensor("idm", [PR, PR], BF16, kind="ExternalInput")
    ctx_o = nc.dram_tensor("ctx_o", [128, NVEC, KD], F32, kind="ExternalOutput")
    # PE-route ctx quarters (time chunks n%4 at PSUM partitions 0/32/64/96);
    # host sums the four.
    ctxq_o = nc.dram_tensor(
        "ctxq_o", [4, BPC - NVEC, DA], F32, kind="ExternalOutput"
    )
    den_o = nc.dram_tensor(
        "den_o", [PR, len(GROUPS), 2], F32, kind="ExternalOutput"
    )

    with tile.TileContext(nc) as tc:
        with tc.tile_pool(name="const", bufs=1) as cpool, tc.tile_pool(
            name="atp", bufs=BPC
        ) as atpool, tc.tile_pool(
            name="anp", bufs=BPC - NVEC
        ) as anpool, tc.tile_pool(name="sb2", bufs=1) as sb2:
            # DMA issue order is the schedule: one HWDGE FIFO ring (Sync).
            # aT tiles stream first, in batch order, split in time-halves so
            # mm1 starts on half A while half B is in flight; a_nat tiles
            # (PE-route ctx data) queue after the whole aT stream.
            at_tiles = []
            for b in range(BPC):
                at_b = atpool.tile([128, KD, TX], BF16, name=f"at{b}", tag="at")
                at_tiles.append(at_b)
            an_tiles = []
            for i in range(BPC - NVEC):
                an_tiles.append(
                    anpool.tile([128, NT, DA], BF16, name=f"an{i}", tag="an")
                )

            nc.sync.dma_start(at_tiles[0][:, :, 0:1024], aT[0, :, :, 0:1024])

            w1a_sb = cpool.tile([128, KD, 64], BF16)
            nc.gpsimd.dma_start(w1a_sb[:], w1a[:])
            w1s_sb = cpool.tile([128, KD, H], F32)
            nc.gpsimd.dma_start(w1s_sb[:], w1s[:])
            sT_sb = cpool.tile([128, KD, BPC], F32)
            nc.gpsimd.dma_start(sT_sb[:], sT[:])
            b1c_sb = cpool.tile([128, 1], F32)
            nc.gpsimd.dma_start(b1c_sb[:], b1c[:])
            w2c_sb = cpool.tile([128, 32], BF16)
            nc.gpsimd.dma_start(w2c_sb[:], w2c[:])
            b2c_sb = cpool.tile([PR, 1], F32)
            nc.gpsimd.dma_start(b2c_sb[:], b2c[:])
            ones_sb = cpool.tile([PR, 128], BF16)
            nc.gpsimd.dma_start(ones_sb[:], ones[:])
            idm_sb = cpool.tile([PR, PR], BF16)
            nc.gpsimd.dma_start(idm_sb[:], idm[:])

            nc.sync.dma_start(at_tiles[0][:, :, 1024:2048], aT[0, :, :, 1024:2048])
            for b in range(1, BPC):
                nc.sync.dma_start(at_tiles[b][:], aT[b])
            for i in range(BPC - NVEC):
                nc.sync.dma_start(an_tiles[i][:], a_nat[i])

            sterm_sb = sb2.tile([128, BPC], F32)
            ctx_sb = sb2.tile([128, NVEC, KD], F32)
            ctxq_sb = sb2.tile([97, BPC - NVEC, DA], F32)
            den_sb = sb2.tile([PR, len(GROUPS), 2], F32)
            dump_sb = sb2.tile([128, 2048], BF16)  # ACT-reduce dummy out

            with tc.tile_pool(name="hps", bufs=2, space=PSUM) as hps, tc.tile_pool(
                name="eps", bufs=1, space=PSUM
            ) as eps, tc.tile_pool(
                name="pbc", bufs=2, space=PSUM
            ) as pbcp, tc.tile_pool(
                name="p3", bufs=2, space=PSUM
            ) as p3, tc.tile_pool(name="hsb", bufs=10) as hsbp, tc.tile_pool(
                name="psb", bufs=2
            ) as psbp, tc.tile_pool(name="tsb", bufs=2) as tsbp, tc.tile_pool(
                name="tmpd", bufs=2
            ) as tmpd, tc.tile_pool(name="ptsb", bufs=1) as ptsbp:
                # PE warm-up: dense dummy matmuls on zeroed scratch keep the
                # PE busy during the initial DMA window (HAM p-state ramp).
                warm_sb = sb2.tile([128, 512], BF16, tag="warm")
                nc.vector.memset(warm_sb[:], 0.0)
                warm_ps = hps.tile([128, 512], F32, tag="hps", name="warm_ps")
                for _ in range(26):
                    nc.tensor.matmul(
                        warm_ps[0:64, :],
                        warm_sb[:, 0:64],
                        warm_sb[:],
                        start=True,
                        stop=True,
                        skip_group_check=True,
                    )
                # s-term, twice: partitions 0-49 (col group 0) and 64-113
                # (col group 64), so both relu halves get a bias.
                nc.gpsimd.memset(sterm_sb[:], 0.0)
                nc.gpsimd.memset(den_sb[:], 0.0)
                sterm_ps = hps.tile([128, BPC], F32, tag="hps")
                for cg in (0, 64):
                    for k in range(KD):
                        nc.tensor.matmul(
                            sterm_ps[cg : cg + H, :],
                            w1s_sb[:, k, :],
                            sT_sb[:, k, :],
                            start=(k == 0),
                            stop=(k == KD - 1),
                            tile_position=(0, cg),
                            skip_group_check=True,
                        )
                    nc.scalar.activation(
                        sterm_sb[cg : cg + H, :],
                        sterm_ps[cg : cg + H, :],
                        AF.Identity,
                        bias=b1c_sb[cg : cg + H, :],
                    )

                # FIFO of deferred PE emitters spliced into later PE stream.
                pending = []

                def drain(n):
                    for _ in range(n):
                        if not pending:
                            return
                        pending.pop(0)()

                def emit_vec_ctx(b, p_sb, j):
                    """Vector-route ctx for batch b: PE-broadcast p row j
                    into PSUM, DVE multiplies, DVE/ACT reduce."""

                    def emit():
                        tmp = tmpd.tile([128, KD, 2048], BF16, tag="tmpd")
                        for q in range(4):
                            pb = pbcp.tile([128, 512], F32, tag="pbc", name="pb")
                            t0 = q * 512
                            nc.tensor.matmul(
                                pb[:],
                                ones_sb[32 * j : 32 * j + 1, :],
                                p_sb[32 * j : 32 * j + 1, t0 : t0 + 512],
                                start=True,
                                stop=True,
                                skip_group_check=True,
                            )
                            for k in range(KD):
                                nc.vector.tensor_tensor(
                                    out=tmp[:, k, t0 : t0 + 512],
                                    in0=at_tiles[b][:, k, t0 : t0 + 512],
                                    in1=pb[:],
                                    op=ALU.mult,
                                )
                        for k in range(KD):
                            if b in ACT_K1 and k == 1:
                                nc.scalar.activation(
                                    dump_sb[:],
                                    tmp[:, k, :],
                                    AF.Identity,
                                    accum_out=ctx_sb[:, b, k : k + 1],
                                )
                            else:
                                nc.vector.tensor_reduce(
                                    out=ctx_sb[:, b, k : k + 1],
                                    in_=tmp[:, k, :],
                                    axis=mybir.AxisListType.X,
                                    op=ALU.add,
                                )

                    pending.append(emit)

                def emit_pe_ctx(b, p_sb, j, pT_sb):
                    """PE-route ctx for batch b (index i in a_nat order)."""
                    i = b - NVEC

                    def emit_tp(n):
                        def emit():
                            pt_ps = p3.tile([128, PR], BF16, tag="p3", name="pt")
                            nc.tensor.transpose(
                                pt_ps[:, 0:PR],
                                p_sb[0:PR, n * 128 : (n + 1) * 128],
                                idm_sb[0:PR, 0:PR],
                            )
                            nc.vector.tensor_copy(
                                pT_sb[:, n, :], pt_ps[:, 0:PR:32]
                            )

                        return emit

                    if j == 0:
                        for n in range(NT):
                            pending.append(emit_tp(n))

                    c_ps = p3.tile([128, DA], F32, tag="p3", name=f"c_ps{b}")

                    def emit_ctx(np_lo):
                        def emit():
                            for np_ in (np_lo, np_lo + 1):
                                for qi, cg in enumerate((0, 32, 64, 96)):
                                    n = 4 * np_ + qi
                                    nc.tensor.matmul(
                                        c_ps[cg : cg + 1, :],
                                        pT_sb[:, n, j : j + 1],
                                        an_tiles[i][:, n, :],
                                        start=(np_ == 0),
                                        stop=(np_ == NT // 4 - 1),
                                        tile_position=(0, cg),
                                        skip_group_check=True,
                                    )

                        return emit

                    for np_lo in range(0, NT // 4, 2):
                        pending.append(emit_ctx(np_lo))

                    def emit_copy():
                        for cg in (0, 32, 64, 96):
                            nc.vector.tensor_copy(
                                ctxq_sb[cg : cg + 1, i, :], c_ps[cg : cg + 1, :]
                            )

                    pending.append(emit_copy)

                for gi, (g0, gsz) in enumerate(GROUPS):
                    # phase 1 (scores mm1): hT = W1a^T @ aT as column-tiled
                    # PAIRS; relu(+s-term bias) on ACT.
                    h_tiles = {}
                    for j in range(gsz):
                        b = g0 + j
                        at_t = at_tiles[b]
                        for tp in range(NTS // 2):
                            h_ps = hps.tile([128, 512], F32, tag="hps")
                            for k in range(KD):
                                for half, cg in enumerate((0, 64)):
                                    ts = 2 * tp + half
                                    nc.tensor.matmul(
                                        h_ps[cg : cg + 64, :],
                                        w1a_sb[:, k, :],
                                        at_t[:, k, ts * 512 : (ts + 1) * 512],
                                        start=(k == 0),
                                        stop=(k == KD - 1),
                                        tile_position=(0, cg),
                                        skip_group_check=True,
                                    )
                            h_sb = hsbp.tile([128, 512], BF16, tag="hsb")
                            h_tiles[(b, tp)] = h_sb
                            nc.scalar.activation(
                                h_sb[:], h_ps[:], AF.Relu, bias=sterm_sb[:, b : b + 1]
                            )
                            # splice deferred ctx work into the mm1 stream
                            drain(2)

                    # phase 2 (scores mm2 + softmax) at the group boundary:
                    # each batch's e row written independently at PSUM
                    # partition 32j; tanh/exp chase each 1024-half.
                    p_sb = psbp.tile([PR, TX], BF16, tag="psb")
                    gr = 32 * gsz
                    for hf in range(2):
                        e_ps = eps.tile([PR, 1024], F32, tag="eps")
                        for sl2 in range(2):
                            sl = 2 * hf + sl2
                            tp, half = sl // 2, sl % 2
                            cg = 64 * half
                            for j in range(gsz):
                                b = g0 + j
                                nc.tensor.matmul(
                                    e_ps[32 * j : 32 * (j + 1), sl2 * 512 : (sl2 + 1) * 512],
                                    w2c_sb[cg : cg + H, :],
                                    h_tiles[(b, tp)][cg : cg + H, :],
                                    start=True,
                                    stop=True,
                                    tile_position=(cg, 32 * j),
                                    skip_group_check=True,
                                )
                        t_sb = tsbp.tile([PR, 1024], F32, tag="tsb")
                        nc.scalar.activation(
                            t_sb[0:gr, :],
                            e_ps[0:gr, :],
                            AF.Tanh,
                            bias=b2c_sb[0:gr, :],
                        )
                        nc.scalar.activation(
                            p_sb[0:gr, hf * 1024 : (hf + 1) * 1024],
                            t_sb[0:gr, :],
                            AF.Exp,
                            accum_out=den_sb[0:gr, gi, hf : hf + 1],
                        )

                    # enqueue phase 3 per batch
                    pT_sb = None
                    if g0 + gsz > NVEC:
                        pT_sb = ptsbp.tile([128, NT, GB], BF16, name=f"pT{gi}")
                    for j in range(gsz):
                        b = g0 + j
                        if b < NVEC:
                            emit_vec_ctx(b, p_sb, j)
                        else:
                            emit_pe_ctx(b, p_sb, j, pT_sb)

                drain(len(pending))
                nc.gpsimd.dma_start(den_o[:], den_sb[:])
                nc.gpsimd.dma_start(ctx_o[:], ctx_sb[:])
                for qi, cg in enumerate((0, 32, 64, 96)):
                    nc.gpsimd.dma_start(
                        ctxq_o[qi], ctxq_sb[cg : cg + 1, :, :]
                    )

    nc.compile()
    return nc


def make_in_maps(a, s, W1, b1, W2, b2):
    a = np.asarray(a, np.float32)
    s = np.asarray(s, np.float32)
    W1 = np.asarray(W1, np.float32)
    b1 = np.asarray(b1, np.float32)
    W2 = np.asarray(W2, np.float32)
    b2 = np.asarray(b2, np.float32)

    a5 = a.reshape(NCORES, BPC, TX, DA)
    s3 = s.reshape(NCORES, BPC, DS)

    w1a_h = np.zeros((128, KD, 64), np.float32)
    w1a_h[:, :, :H] = W1[:DA].reshape(KD, 128, H).transpose(1, 0, 2)
    w1a_h = w1a_h.astype(NPBF16)
    w1s_h = np.ascontiguousarray(
        W1[DA:].reshape(KD, 128, H).transpose(1, 0, 2)
    ).astype(np.float32)
    b1c_h = np.zeros((128, 1), np.float32)
    b1c_h[0:H, 0] = b1
    b1c_h[64 : 64 + H, 0] = b1
    w2c_h = np.zeros((128, 32), np.float32)
    w2c_h[0:H, 0] = W2[:, 0]
    w2c_h[64 : 64 + H, 0] = W2[:, 0]
    w2c_h = w2c_h.astype(NPBF16)
    b2c_h = np.full((PR, 1), float(b2.reshape(-1)[0]), np.float32)
    ones_h = np.ones((PR, 128), NPBF16)
    idm_h = np.eye(PR).astype(NPBF16)

    in_maps = []
    for i in range(NCORES):
        ai = a5[i]
        aT_h = np.ascontiguousarray(
            ai.transpose(0, 2, 1)
            .reshape(BPC, KD, 128, TX)
            .transpose(0, 2, 1, 3)
        ).astype(NPBF16)
        a_nat_h = np.ascontiguousarray(
            ai[NVEC:].reshape(BPC - NVEC, NT, 128, DA).transpose(0, 2, 1, 3)
        ).astype(NPBF16)
        sT_h = np.ascontiguousarray(
            s3[i].T.reshape(KD, 128, BPC).transpose(1, 0, 2)
        ).astype(np.float32)
        in_maps.append(
            {
                "aT": aT_h,
                "a_nat": a_nat_h,
                "w1a": w1a_h,
                "w1s": w1s_h,
                "sT": sT_h,
                "b1c": b1c_h,
                "w2c": w2c_h,
                "b2c": b2c_h,
                "ones": ones_h,
                "idm": idm_h,
            }
        )
    return in_maps


def assemble_output(results):
    outs = []
    for i in range(NCORES):
        r = results[i]
        ctx = r["ctx_o"].astype(np.float64)  # [128, NVEC, KD]
        ctxq = r["ctxq_o"].astype(np.float64)  # [4, BPC-NVEC, DA]
        den4 = r["den_o"].astype(np.float64)  # [PR, n_groups, 2]
        full = np.empty((BPC, KD * 128), np.float64)
        full[:NVEC] = ctx.transpose(1, 2, 0).reshape(NVEC, KD * 128)
        full[NVEC:] = ctxq.sum(axis=0)
        den = np.empty((BPC, 1), np.float64)
        for gi, (g0, gsz) in enumerate(GROUPS):
            for j in range(gsz):
                den[g0 + j, 0] = den4[32 * j, gi, :].sum()
        outs.append(full / den)
    return np.concatenate(outs, 0).reshape(B, 1, DA).astype(np.float32)


_NC_CACHE = None


def _get_nc():
    global _NC_CACHE
    if _NC_CACHE is None:
        _NC_CACHE = build_nc()
    return _NC_CACHE


def kernel(a, s, W1, b1, W2, b2, trace=False):
    from concourse.bass_utils import run_bass_kernel_spmd

    nc = _get_nc()
    in_maps = make_in_maps(a, s, W1, b1, W2, b2)
    res = run_bass_kernel_spmd(
        nc, in_maps, core_ids=list(range(NCORES)), trace=trace
    )
    out = assemble_output(res.results)
    if trace:
        kernel.last_exec_time_ns = res.exec_time_ns
        kernel.last_results = res
    return out


# revision 22
# speedup vs baseline: 1.3189x; 1.3189x over previous
"""Trainium2 Bass kernel for nn_AttentionLayer (Bahdanau-style attention scorer).

Math (per batch b):
    x   = concat([a, broadcast(s)], -1)            # [Tx, Da+Ds]
    h   = relu(x @ W1 + b1)                        # [Tx, H]
    e   = tanh(h @ W2 + b2)                        # [Tx, 1]
    al  = softmax(e, axis=Tx)
    ctx = al^T @ a                                 # [1, Da]

Since e = tanh(.) is in [-1, 1], softmax needs no max subtraction.

Sharding: data-parallel over B across 8 cores (8 batches each).

v3 design — 1.5-layout streaming (12.6 MB/core vs the baseline's 16.8):
`a` ships transposed+bf16 (aT: features on partitions) for ALL batches —
the score matmuls consume it directly — and ADDITIONALLY in natural
layout (a_nat) for batches 4-7 only, queued AFTER the aT stream.

Context path is split by batch to balance engines:
  - batches 0-3 ("vector route", softmax output available early): PE
    broadcasts the batch's softmax row p into PSUM fp32 (K=1 matmuls
    with a ones stationary); DVE multiplies aT slices against it into a
    bf16 scratch; the time reduction runs as DVE tensor_reduce (k=0) and
    ACT Identity+accum_out (k=1 of batches 2-3) to share the load.
    (The fused DVE tensor_tensor_reduce op crashes this runtime, and
    gpsimd cannot touch PSUM — hence this split.)
  - batches 4-7 ("PE route"): p transposed to time-major via PE
    transposes, then ctx = sum_n pT_n^T @ a_nat_n as 4-way column-tiled
    quads accumulating at PSUM partitions 0/32/64/96 (quarters summed on
    host), exactly while their a_nat tiles stream in.

Scores: mm1 as column-tiled PAIRS (two 512-wide time slices through
array cols 0-63/64-127); relu+s-term bias on ACT (s @ W1s + b1 is
time-independent, folded into the bias).  mm2 writes each batch's e row
independently at PSUM partition 32j via tile_position column quadrants
(M=32 stationary with W2 in col 0 zero-pads rows 32j+1..32j+31 so
tanh/exp can run over a contiguous initialized block).  Softmax groups
(2,2,4); tanh/exp in 1024-halves on ACT with accum_out partial
denominators (summed on host; division on host).

Host-side preprocessing (transpose/cast/shard + final division) is numpy.
"""

import os
import sys

import numpy as np

for _p in ("/opt/trn_rl_repo", "/root/.axon_site/_ro/trn_rl_repo"):
    if os.path.isdir(_p) and _p not in sys.path:
        sys.path.insert(0, _p)

import ml_dtypes  # noqa: E402

import concourse.bacc as bacc  # noqa: E402
import concourse.bass as bass  # noqa: E402
import concourse.mybir as mybir  # noqa: E402
import concourse.tile as tile  # noqa: E402

BF16 = mybir.dt.bfloat16
F32 = mybir.dt.float32
NPBF16 = ml_dtypes.bfloat16
AF = mybir.ActivationFunctionType
ALU = mybir.AluOpType
PSUM = bass.MemorySpace.PSUM

NCORES = 8
B, TX, DA, DS, H = 64, 2048, 256, 256, 50
BPC = B // NCORES  # batches per core
NT = TX // 128  # 128-wide time chunks (PE-route ctx)
NTS = TX // 512  # 512-wide time slices
KD = DA // 128  # feature chunks of a

GROUPS = [(0, 2), (2, 2), (4, 4)]  # (first batch, size) per softmax group
GB = max(sz for _, sz in GROUPS)
PR = 32 * GB  # e/p rows live at partitions 0/32/64/96 (base-partition rule)

NVEC = 2  # batches 0..NVEC-1 take the vector route; the rest the PE route
ACT_K1 = ()  # vector-route batches whose k=1 reduction runs on ACT (ACT is busy)


def build_nc():
    """Build the (SPMD-identical) single-core Bass program."""
    nc = bacc.Bacc(
        "TRN2", target_bir_lowering=False, debug=False, num_devices=NCORES
    )

    aT = nc.dram_tensor("aT", [BPC, 128, KD, TX], BF16, kind="ExternalInput")
    a_nat = nc.dram_tensor(
        "a_nat", [BPC - NVEC, 128, NT, DA], BF16, kind="ExternalInput"
    )
    w1a = nc.dram_tensor("w1a", [128, KD, 64], BF16, kind="ExternalInput")
    w1s = nc.dram_tensor("w1s", [128, KD, H], F32, kind="ExternalInput")
    sT = nc.dram_tensor("sT", [128, KD, BPC], F32, kind="ExternalInput")
    # b1c / w2c carry two copies of their payload: partition rows 0-49 and
    # 64-113 (the two tile_position column/row groups used below).
    b1c = nc.dram_tensor("b1c", [128, 1], F32, kind="ExternalInput")
    w2c = nc.dram_tensor("w2c", [128, 32], BF16, kind="ExternalInput")
    b2c = nc.dram_tensor("b2c", [PR, 1], F32, kind="ExternalInput")
    ones = nc.dram_tensor("ones", [PR, 128], BF16, kind="ExternalInput")
    idm = nc.dram_tensor("idm", [PR, PR], BF16, kind="ExternalInput")
    ctx_o = nc.dram_tensor("ctx_o", [128, NVEC, KD], F32, kind="ExternalOutput")
    # PE-route ctx quarters (time chunks n%4 at PSUM partitions 0/32/64/96);
    # host sums the four.
    ctxq_o = nc.dram_tensor(
        "ctxq_o", [4, BPC - NVEC, DA], F32, kind="ExternalOutput"
    )
    den_o = nc.dram_tensor(
        "den_o", [PR, len(GROUPS), 2], F32, kind="ExternalOutput"
    )

    with tile.TileContext(nc) as tc:
        with tc.tile_pool(name="const", bufs=1) as cpool, tc.tile_pool(
            name="atp", bufs=BPC
        ) as atpool, tc.tile_pool(
            name="anp", bufs=BPC - NVEC
        ) as anpool, tc.tile_pool(name="sb2", bufs=1) as sb2:
            # DMA issue order is the schedule: one HWDGE FIFO ring (Sync).
            # aT tiles stream first, in batch order, split in time-halves so
            # mm1 starts on half A while half B is in flight; a_nat tiles
            # (PE-route ctx data) queue after the whole aT stream.
            at_tiles = []
            for b in range(BPC):
                at_b = atpool.tile([128, KD, TX], BF16, name=f"at{b}", tag="at")
                at_tiles.append(at_b)
            an_tiles = []
            for i in range(BPC - NVEC):
                an_tiles.append(
                    anpool.tile([128, NT, DA], BF16, name=f"an{i}", tag="an")
                )

            nc.sync.dma_start(at_tiles[0][:, :, 0:1024], aT[0, :, :, 0:1024])

            w1a_sb = cpool.tile([128, KD, 64], BF16)
            nc.gpsimd.dma_start(w1a_sb[:], w1a[:])
            w1s_sb = cpool.tile([128, KD, H], F32)
            nc.gpsimd.dma_start(w1s_sb[:], w1s[:])
            sT_sb = cpool.tile([128, KD, BPC], F32)
            nc.gpsimd.dma_start(sT_sb[:], sT[:])
            b1c_sb = cpool.tile([128, 1], F32)
            nc.gpsimd.dma_start(b1c_sb[:], b1c[:])
            w2c_sb = cpool.tile([128, 32], BF16)
            nc.gpsimd.dma_start(w2c_sb[:], w2c[:])
            b2c_sb = cpool.tile([PR, 1], F32)
            nc.gpsimd.dma_start(b2c_sb[:], b2c[:])
            ones_sb = cpool.tile([PR, 128], BF16)
            nc.gpsimd.dma_start(ones_sb[:], ones[:])
            idm_sb = cpool.tile([PR, PR], BF16)
            nc.gpsimd.dma_start(idm_sb[:], idm[:])

            nc.sync.dma_start(at_tiles[0][:, :, 1024:2048], aT[0, :, :, 1024:2048])
            for b in range(1, 4):
                nc.sync.dma_start(at_tiles[b][:], aT[b])
            for i in (0, 1):  # a_nat for batches 2, 3
                nc.sync.dma_start(an_tiles[i][:], a_nat[i])
            for b in range(4, BPC):
                nc.sync.dma_start(at_tiles[b][:], aT[b])
            for i in range(2, BPC - NVEC):
                nc.sync.dma_start(an_tiles[i][:], a_nat[i])

            sterm_sb = sb2.tile([128, BPC], F32)
            ctx_sb = sb2.tile([128, NVEC, KD], F32)
            ctxq_sb = sb2.tile([97, BPC - NVEC, DA], F32)
            den_sb = sb2.tile([PR, len(GROUPS), 2], F32)
            dump_sb = sb2.tile([128, 2048], BF16)  # ACT-reduce dummy out

            with tc.tile_pool(name="hps", bufs=2, space=PSUM) as hps, tc.tile_pool(
                name="eps", bufs=1, space=PSUM
            ) as eps, tc.tile_pool(
                name="pbc", bufs=2, space=PSUM
            ) as pbcp, tc.tile_pool(
                name="p3", bufs=2, space=PSUM
            ) as p3, tc.tile_pool(name="hsb", bufs=10) as hsbp, tc.tile_pool(
                name="psb", bufs=2
            ) as psbp, tc.tile_pool(name="tsb", bufs=2) as tsbp, tc.tile_pool(
                name="tmpd", bufs=2
            ) as tmpd, tc.tile_pool(name="ptsb", bufs=1) as ptsbp:
                # PE warm-up: dense dummy matmuls on zeroed scratch keep the
                # PE busy during the initial DMA window (HAM p-state ramp).
                warm_sb = sb2.tile([128, 512], BF16, tag="warm")
                nc.vector.memset(warm_sb[:], 0.0)
                warm_ps = hps.tile([128, 512], F32, tag="hps", name="warm_ps")
                for _ in range(26):
                    nc.tensor.matmul(
                        warm_ps[0:64, :],
                        warm_sb[:, 0:64],
                        warm_sb[:],
                        start=True,
                        stop=True,
                        skip_group_check=True,
                    )
                # s-term, twice: partitions 0-49 (col group 0) and 64-113
                # (col group 64), so both relu halves get a bias.
                nc.gpsimd.memset(sterm_sb[:], 0.0)
                nc.gpsimd.memset(den_sb[:], 0.0)
                sterm_ps = hps.tile([128, BPC], F32, tag="hps")
                for cg in (0, 64):
                    for k in range(KD):
                        nc.tensor.matmul(
                            sterm_ps[cg : cg + H, :],
                            w1s_sb[:, k, :],
                            sT_sb[:, k, :],
                            start=(k == 0),
                            stop=(k == KD - 1),
                            tile_position=(0, cg),
                            skip_group_check=True,
                        )
                    nc.scalar.activation(
                        sterm_sb[cg : cg + H, :],
                        sterm_ps[cg : cg + H, :],
                        AF.Identity,
                        bias=b1c_sb[cg : cg + H, :],
                    )

                # FIFO of deferred PE emitters spliced into later PE stream.
                pending = []

                def drain(n):
                    for _ in range(n):
                        if not pending:
                            return
                        pending.pop(0)()

                def emit_vec_ctx(b, p_sb, j):
                    """Vector-route ctx for batch b: PE-broadcast p row j
                    into PSUM, DVE multiplies, DVE/ACT reduce."""

                    def emit():
                        tmp = tmpd.tile([128, KD, 2048], BF16, tag="tmpd")
                        for q in range(4):
                            pb = pbcp.tile([128, 512], F32, tag="pbc", name="pb")
                            t0 = q * 512
                            nc.tensor.matmul(
                                pb[:],
                                ones_sb[32 * j : 32 * j + 1, :],
                                p_sb[32 * j : 32 * j + 1, t0 : t0 + 512],
                                start=True,
                                stop=True,
                                skip_group_check=True,
                            )
                            for k in range(KD):
                                nc.vector.tensor_tensor(
                                    out=tmp[:, k, t0 : t0 + 512],
                                    in0=at_tiles[b][:, k, t0 : t0 + 512],
                                    in1=pb[:],
                                    op=ALU.mult,
                                )
                        for k in range(KD):
                            if b in ACT_K1 and k == 1:
                                nc.scalar.activation(
                                    dump_sb[:],
                                    tmp[:, k, :],
                                    AF.Identity,
                                    accum_out=ctx_sb[:, b, k : k + 1],
                                )
                            else:
                                nc.vector.tensor_reduce(
                                    out=ctx_sb[:, b, k : k + 1],
                                    in_=tmp[:, k, :],
                                    axis=mybir.AxisListType.X,
                                    op=ALU.add,
                                )

                    pending.append(emit)

                def emit_pe_ctx(b, p_sb, j, pT_sb):
                    """PE-route ctx for batch b (index i in a_nat order)."""
                    i = b - NVEC

                    gs = gsz_of[b]
                    gr = 32 * (gs - 1) + 1

                    def emit_tp(q):
                        def emit():
                            pt_ps = p3.tile([128, 4, PR], BF16, tag="p3", name="pt")
                            for c in range(4):
                                n = 4 * q + c
                                nc.tensor.transpose(
                                    pt_ps[:, c, 0:gr],
                                    p_sb[0:gr, n * 128 : (n + 1) * 128],
                                    idm_sb[0:gr, 0:gr],
                                )
                            nc.vector.tensor_copy(
                                pT_sb[:, 4 * q : 4 * q + 4, 0:gs],
                                pt_ps[:, :, 0:gr:32],
                            )

                        return emit

                    if j == 0:
                        for q in range(NT // 4):
                            pending.append(emit_tp(q))

                    c_ps = p3.tile([128, DA], F32, tag="p3", name=f"c_ps{b}")

                    def emit_ctx(np_lo):
                        def emit():
                            for np_ in (np_lo, np_lo + 1):
                                for qi, cg in enumerate((0, 32, 64, 96)):
                                    n = 4 * np_ + qi
                                    nc.tensor.matmul(
                                        c_ps[cg : cg + 1, :],
                                        pT_sb[:, n, j : j + 1],
                                        an_tiles[i][:, n, :],
                                        start=(np_ == 0),
                                        stop=(np_ == NT // 4 - 1),
                                        tile_position=(0, cg),
                                        skip_group_check=True,
                                    )

                        return emit

                    for np_lo in range(0, NT // 4, 2):
                        pending.append(emit_ctx(np_lo))

                    def emit_out():
                        for qi, cg in enumerate((0, 32, 64, 96)):
                            if qi % 2 == 0:
                                nc.vector.tensor_copy(
                                    ctxq_sb[cg : cg + 1, i, :],
                                    c_ps[cg : cg + 1, :],
                                )
                            else:
                                nc.scalar.activation(
                                    ctxq_sb[cg : cg + 1, i, :],
                                    c_ps[cg : cg + 1, :],
                                    AF.Identity,
                                )

                    pending.append(emit_out)

                gsz_of = {}
                for g0_, gsz_ in GROUPS:
                    for j_ in range(gsz_):
                        gsz_of[g0_ + j_] = gsz_

                for gi, (g0, gsz) in enumerate(GROUPS):
                    # phase 1 (scores mm1): hT = W1a^T @ aT as column-tiled
                    # PAIRS; relu(+s-term bias) on ACT.
                    h_tiles = {}
                    for j in range(gsz):
                        b = g0 + j
                        at_t = at_tiles[b]
                        for tp in range(NTS // 2):
                            h_ps = hps.tile([128, 512], F32, tag="hps")
                            for k in range(KD):
                                for half, cg in enumerate((0, 64)):
                                    ts = 2 * tp + half
                                    nc.tensor.matmul(
                                        h_ps[cg : cg + 64, :],
                                        w1a_sb[:, k, :],
                                        at_t[:, k, ts * 512 : (ts + 1) * 512],
                                        start=(k == 0),
                                        stop=(k == KD - 1),
                                        tile_position=(0, cg),
                                        skip_group_check=True,
                                    )
                            h_sb = hsbp.tile([128, 512], BF16, tag="hsb")
                            h_tiles[(b, tp)] = h_sb
                            nc.scalar.activation(
                                h_sb[:], h_ps[:], AF.Relu, bias=sterm_sb[:, b : b + 1]
                            )
                            # splice deferred ctx work into the mm1 stream
                            drain(2)

                    # phase 2 (scores mm2 + softmax) at the group boundary:
                    # each batch's e row written independently at PSUM
                    # partition 32j; tanh/exp chase each 1024-half.
                    p_sb = psbp.tile([PR, TX], BF16, tag="psb")
                    gr = 32 * gsz
                    for hf in range(2):
                        e_ps = eps.tile([PR, 1024], F32, tag="eps")
                        for sl2 in range(2):
                            sl = 2 * hf + sl2
                            tp, half = sl // 2, sl % 2
                            cg = 64 * half
                            for j in range(gsz):
                                b = g0 + j
                                nc.tensor.matmul(
                                    e_ps[32 * j : 32 * (j + 1), sl2 * 512 : (sl2 + 1) * 512],
                                    w2c_sb[cg : cg + H, :],
                                    h_tiles[(b, tp)][cg : cg + H, :],
                                    start=True,
                                    stop=True,
                                    tile_position=(cg, 32 * j),
                                    skip_group_check=True,
                                )
                        t_sb = tsbp.tile([PR, 1024], F32, tag="tsb")
                        nc.scalar.activation(
                            t_sb[0:gr, :],
                            e_ps[0:gr, :],
                            AF.Tanh,
                            bias=b2c_sb[0:gr, :],
                        )
                        nc.scalar.activation(
                            p_sb[0:gr, hf * 1024 : (hf + 1) * 1024],
                            t_sb[0:gr, :],
                            AF.Exp,
                            accum_out=den_sb[0:gr, gi, hf : hf + 1],
                        )

                    # enqueue phase 3 per batch
                    pT_sb = None
                    if g0 + gsz > NVEC:
                        pT_sb = ptsbp.tile([128, NT, GB], BF16, name=f"pT{gi}")
                    for j in range(gsz):
                        b = g0 + j
                        if b < NVEC:
                            emit_vec_ctx(b, p_sb, j)
                        else:
                            emit_pe_ctx(b, p_sb, j, pT_sb)

                drain(len(pending))
                nc.gpsimd.dma_start(den_o[:], den_sb[:])
                nc.gpsimd.dma_start(ctx_o[:], ctx_sb[:])
                for qi, cg in enumerate((0, 32, 64, 96)):
                    nc.gpsimd.dma_start(ctxq_o[qi], ctxq_sb[cg : cg + 1, :, :])

    nc.compile()
    return nc


def make_in_maps(a, s, W1, b1, W2, b2):
    a = np.asarray(a, np.float32)
    s = np.asarray(s, np.float32)
    W1 = np.asarray(W1, np.float32)
    b1 = np.asarray(b1, np.float32)
    W2 = np.asarray(W2, np.float32)
    b2 = np.asarray(b2, np.float32)

    a5 = a.reshape(NCORES, BPC, TX, DA)
    s3 = s.reshape(NCORES, BPC, DS)

    w1a_h = np.zeros((128, KD, 64), np.float32)
    w1a_h[:, :, :H] = W1[:DA].reshape(KD, 128, H).transpose(1, 0, 2)
    w1a_h = w1a_h.astype(NPBF16)
    w1s_h = np.ascontiguousarray(
        W1[DA:].reshape(KD, 128, H).transpose(1, 0, 2)
    ).astype(np.float32)
    b1c_h = np.zeros((128, 1), np.float32)
    b1c_h[0:H, 0] = b1
    b1c_h[64 : 64 + H, 0] = b1
    w2c_h = np.zeros((128, 32), np.float32)
    w2c_h[0:H, 0] = W2[:, 0]
    w2c_h[64 : 64 + H, 0] = W2[:, 0]
    w2c_h = w2c_h.astype(NPBF16)
    b2c_h = np.full((PR, 1), float(b2.reshape(-1)[0]), np.float32)
    ones_h = np.ones((PR, 128), NPBF16)
    idm_h = np.eye(PR).astype(NPBF16)

    in_maps = []
    for i in range(NCORES):
        ai = a5[i]
        aT_h = np.ascontiguousarray(
            ai.transpose(0, 2, 1)
            .reshape(BPC, KD, 128, TX)
            .transpose(0, 2, 1, 3)
        ).astype(NPBF16)
        a_nat_h = np.ascontiguousarray(
            ai[NVEC:].reshape(BPC - NVEC, NT, 128, DA).transpose(0, 2, 1, 3)
        ).astype(NPBF16)
        sT_h = np.ascontiguousarray(
            s3[i].T.reshape(KD, 128, BPC).transpose(1, 0, 2)
        ).astype(np.float32)
        in_maps.append(
            {
                "aT": aT_h,
                "a_nat": a_nat_h,
                "w1a": w1a_h,
                "w1s": w1s_h,
                "sT": sT_h,
                "b1c": b1c_h,
                "w2c": w2c_h,
                "b2c": b2c_h,
                "ones": ones_h,
                "idm": idm_h,
            }
        )
    return in_maps


def assemble_output(results):
    outs = []
    for i in range(NCORES):
        r = results[i]
        ctx = r["ctx_o"].astype(np.float64)  # [128, NVEC, KD]
        ctxq = r["ctxq_o"].astype(np.float64)  # [4, BPC-NVEC, DA]
        den4 = r["den_o"].astype(np.float64)  # [PR, n_groups, 2]
        full = np.empty((BPC, KD * 128), np.float64)
        full[:NVEC] = ctx.transpose(1, 2, 0).reshape(NVEC, KD * 128)
        full[NVEC:] = ctxq.sum(axis=0)
        den = np.empty((BPC, 1), np.float64)
        for gi, (g0, gsz) in enumerate(GROUPS):
            for j in range(gsz):
                den[g0 + j, 0] = den4[32 * j, gi, :].sum()
        outs.append(full / den)
    return np.concatenate(outs, 0).reshape(B, 1, DA).astype(np.float32)


_NC_CACHE = None


def _get_nc():
    global _NC_CACHE
    if _NC_CACHE is None:
        _NC_CACHE = build_nc()
    return _NC_CACHE


def kernel(a, s, W1, b1, W2, b2, trace=False):
    from concourse.bass_utils import run_bass_kernel_spmd

    nc = _get_nc()
    in_maps = make_in_maps(a, s, W1, b1, W2, b2)
    res = run_bass_kernel_spmd(
        nc, in_maps, core_ids=list(range(NCORES)), trace=trace
    )
    out = assemble_output(res.results)
    if trace:
        kernel.last_exec_time_ns = res.exec_time_ns
        kernel.last_results = res
    return out
